# revision 6
# baseline (speedup 1.0000x reference)
"""GAT (3-layer) on 8 TRN2 NeuronCores.

Strategy (dst-sharded, graph-parallel):
- Nodes sharded 8 ways (5000 -> padded 5120/core). Edges sharded by dst owner,
  sorted by (dst tile, src), split lo/hi on src for int16 dma_gather indices.
- Per layer per core: z_aug = h_loc @ [W1 | W1@wa1 | W1@wa2] (+ z_i = h@W2)
  via PE; AllGather z rows -> full table [40960, 192] in each core's HBM;
  per 128-edge block: dma_gather rows [z|1|s1], one-hot M matmuls for
  s2 broadcast + softmax-weighted aggregation (denominator via the ones
  column); h_new = relu(z_i + z_nb/denom).
- Segment max is skipped (logits are small; exp can't overflow) -> exact
  softmax up to fp rounding. Zero-degree nodes handled by denom floor.
"""
import sys
sys.path.insert(0, "/opt/trn_rl_repo")
import numpy as np

import concourse.bass as bass
import concourse.bacc as bacc
import concourse.tile as tile
import concourse.mybir as mybir
from concourse.bass_utils import run_bass_kernel_spmd
from concourse.masks import make_identity

NC = 8
P = 128
N, E, D, L = 40000, 640000, 128, 3
SH, SHP = 5000, 5120          # nodes per shard, padded
NT = SHP * NC                 # 40960 padded global nodes
TPC = SHP // P                # 40 tiles per core
R = 192                       # table row stride in floats (768B, 3*256)
LO = 32768                    # lo/hi table split for int16 indices
F32 = mybir.dt.float32
I16 = mybir.dt.int16
AOT = mybir.AluOpType
ACT = mybir.ActivationFunctionType


def _pad_idx(g):
    """global node id -> padded global row"""
    sh = g // SH
    return sh * SHP + (g - sh * SH)


def preprocess(src, dst, d):
    """Sort/shard/pad edges. Returns per-core arrays + shared block schedule."""
    srcp = _pad_idx(src)
    dstp_g = _pad_idx(dst)
    owner = dstp_g // SHP

    # trash rows: z=0 and s1=-1e6 (set on device): lo -> row 5000 (core0 pad),
    # hi -> row 40959 (core7 last pad row; 40959-32768=8191 fits int16)
    TR_LO, TR_HI = SH, NT - 1

    per_core = []
    for c in range(NC):
        m = owner == c
        per_core.append((srcp[m], dstp_g[m] - c * SHP, d[m]))

    # group by (tile, half); block counts shared across cores (max)
    B_lo = np.zeros(TPC, np.int64)
    B_hi = np.zeros(TPC, np.int64)
    grouped = []  # [core][tile] -> (lo_s, lo_d, lo_dl, hi_s, hi_d, hi_dl)
    for c in range(NC):
        s, dl, dv = per_core[c]
        t = dl // P
        tiles = []
        for ti in range(TPC):
            mt = t == ti
            st, dlt, dvt = s[mt], dl[mt] - ti * P, dv[mt]
            lo = st < LO
            o_lo = np.argsort(st[lo], kind="stable")
            o_hi = np.argsort(st[~lo], kind="stable")
            tiles.append((st[lo][o_lo], dlt[lo][o_lo], dvt[lo][o_lo],
                          st[~lo][o_hi], dlt[~lo][o_hi], dvt[~lo][o_hi]))
            B_lo[ti] = max(B_lo[ti], (len(o_lo) + P - 1) // P)
            B_hi[ti] = max(B_hi[ti], (len(o_hi) + P - 1) // P)
        grouped.append(tiles)
    B_lo = np.maximum(B_lo, 1)
    B_hi = np.maximum(B_hi, 1)
    NBLK = int((B_lo + B_hi).sum())

    # build flat per-core arrays in schedule order
    idx_cols = NBLK * P // 16
    idx16 = np.zeros((NC, P, idx_cols), np.int16)
    dstp = np.zeros((NC, P, NBLK), np.float32)
    dcol = np.zeros((NC, P, NBLK), np.float32)
    for c in range(NC):
        blk = 0
        for ti in range(TPC):
            ls, ld, lv, hs, hd, hv = grouped[c][ti]
            for (ss, dd, vv, Bn, trash, base) in (
                    (ls, ld, lv, int(B_lo[ti]), TR_LO, 0),
                    (hs, hd, hv, int(B_hi[ti]), TR_HI, LO)):
                npad = Bn * P
                si = np.full(npad, trash, np.int64)
                di = np.zeros(npad, np.int64)
                vi = np.zeros(npad, np.float32)
                si[:len(ss)] = ss
                di[:len(dd)] = dd
                vi[:len(vv)] = vv
                w = (si - base).astype(np.int16).reshape(npad // 16, 16).T
                idx16[c, :, blk * 8: blk * 8 + npad // 16] = np.tile(w, (8, 1))
                dstp[c, :, blk:blk + Bn] = di.reshape(Bn, P).T
                dcol[c, :, blk:blk + Bn] = vi.reshape(Bn, P).T
                blk += Bn
    return B_lo, B_hi, NBLK, idx16, dstp, dcol


def build_nc(B_lo, B_hi, NBLK):
    nc = bacc.Bacc("TRN2", target_bir_lowering=False, debug=False,
                   enable_asserts=False, num_devices=NC)
    h0 = nc.dram_tensor("h0", [SHP, D], F32, kind="ExternalInput")
    waug = nc.dram_tensor("waug", [D, L * (2 * D + 2)], F32, kind="ExternalInput")
    c0b = nc.dram_tensor("c0b", [P, L], F32, kind="ExternalInput")
    idx16 = nc.dram_tensor("idx16", [P, NBLK * P // 16], I16, kind="ExternalInput")
    dstp = nc.dram_tensor("dstp", [P, NBLK], F32, kind="ExternalInput")
    dcol = nc.dram_tensor("dcol", [P, NBLK], F32, kind="ExternalInput")
    hout = nc.dram_tensor("hout", [SHP, D], F32, kind="ExternalOutput")

    zshard = nc.dram_tensor("zshard", [SHP, R], F32, kind="Internal")
    table = nc.dram_tensor("table", [NT, R], F32, kind="Internal",
                           addr_space="Shared")
    MAXB = int(max(B_lo.max(), B_hi.max()))
    MAXTB = int((B_lo + B_hi).max())

    with tile.TileContext(nc) as tc:
        with (
            tc.tile_pool(name="const", bufs=1) as cpool,
            tc.tile_pool(name="sbuf", bufs=3) as sbuf,
            tc.tile_pool(name="hcur", bufs=1) as hcur_p,
            tc.tile_pool(name="hnew", bufs=1) as hnew_p,
            tc.tile_pool(name="zi", bufs=1) as zi_p,
            tc.tile_pool(name="gring", bufs=3) as gring,
            tc.tile_pool(name="blkpool", bufs=4) as blkp,
            tc.tile_pool(name="ps_tr", bufs=2, space="PSUM") as ps_tr,
            tc.tile_pool(name="ps_za", bufs=2, space="PSUM") as ps_za,
            tc.tile_pool(name="ps_s2", bufs=2, space="PSUM") as ps_s2,
            tc.tile_pool(name="ps_ag", bufs=2, space="PSUM") as ps_ag,
        ):
            # ---- constants ----
            ident = cpool.tile([P, P], F32, tag="ident")
            make_identity(nc, ident[:])
            iota_i = cpool.tile([P, P], mybir.dt.int32, tag="iota_i")
            nc.gpsimd.iota(iota_i[:], pattern=[[1, P]], base=0, channel_multiplier=0)
            iota_row = cpool.tile([P, P], F32, tag="iota_row")
            nc.vector.tensor_copy(iota_row[:], iota_i[:])
            iota_ci = cpool.tile([P, 1], mybir.dt.int32, tag="iota_ci")
            nc.gpsimd.iota(iota_ci[:], pattern=[[1, 1]], base=0, channel_multiplier=1)
            iota_col = cpool.tile([P, 1], F32, tag="iota_col")
            nc.vector.tensor_copy(iota_col[:], iota_ci[:])
            # pad-row s1 mask for the last tile: -1e6 on partitions >= 8
            padmask = cpool.tile([P, 1], F32, tag="padmask")
            nc.vector.tensor_scalar(out=padmask[:], in0=iota_col[:],
                                    scalar1=float(SH - (TPC - 1) * P) - 0.5,
                                    scalar2=-1.0e6,
                                    op0=AOT.is_ge, op1=AOT.mult)

            waug_t = cpool.tile([P, L * (2 * D + 2)], F32, tag="waug")
            nc.sync.dma_start(waug_t[:], waug[:, :])
            c0_t = cpool.tile([P, L], F32, tag="c0")
            nc.sync.dma_start(c0_t[:], c0b[:])
            idx_t = cpool.tile([P, NBLK * P // 16], I16, tag="idx")
            nc.sync.dma_start(idx_t[:], idx16[:])
            dstp_t = cpool.tile([P, NBLK], F32, tag="dstp")
            nc.sync.dma_start(dstp_t[:], dstp[:])
            tcol_t = cpool.tile([P, NBLK], F32, tag="tcol")  # d * c0 per layer

            # s2 per local node, per tile column
            s2sb = cpool.tile([P, TPC], F32, tag="s2sb")

            # ---- load h0 ----
            h_tiles = []
            for t in range(TPC):
                ht = hcur_p.tile([P, D], F32, tag=f"h{t}")
                nc.sync.dma_start(ht[:], h0[t * P:(t + 1) * P, :])
                h_tiles.append(ht)

            dcol_t = cpool.tile([P, NBLK], F32, tag="dcol")
            nc.sync.dma_start(dcol_t[:], dcol[:])

            for layer in range(L):
                w_off = layer * (2 * D + 2)
                # t = d * c0[layer]
                nc.vector.tensor_scalar_mul(
                    tcol_t[:], dcol_t[:], c0_t[:, layer:layer + 1])

                # ---- z_aug production per tile ----
                zi_tiles = []
                for t in range(TPC):
                    trp = ps_tr.tile([P, P], F32, tag="tr")
                    nc.tensor.transpose(out=trp[:], in_=h_tiles[t][:],
                                        identity=ident[:])
                    hT = sbuf.tile([P, P], F32, tag="hT")
                    nc.scalar.copy(hT[:], trp[:])
                    zap = ps_za.tile([P, 2 * D + 2], F32, tag="za")
                    nc.tensor.matmul(zap[:], hT[:],
                                     waug_t[:, w_off:w_off + 2 * D + 2],
                                     start=True, stop=True)
                    # staging row = [z(128) | 1.0 | s1 | junk..]
                    stg = sbuf.tile([P, R], F32, tag="stg")
                    nc.scalar.copy(stg[:, 0:D], zap[:, 0:D])
                    nc.vector.memset(stg[:, D:D + 1], 1.0)
                    nc.scalar.copy(stg[:, D + 1:D + 2], zap[:, D:D + 1])
                    if t == TPC - 1:
                        # pad rows (incl. trash row): s1 += -1e6
                        nc.vector.tensor_add(stg[:, D + 1:D + 2],
                                             stg[:, D + 1:D + 2], padmask[:])
                    nc.scalar.copy(s2sb[:, t:t + 1], zap[:, D + 1:D + 2])
                    zi = zi_p.tile([P, D], F32, tag=f"zi{t}")
                    nc.scalar.copy(zi[:], zap[:, D + 2:2 * D + 2])
                    zi_tiles.append(zi)
                    nc.sync.dma_start(zshard[t * P:(t + 1) * P, :], stg[:])

                # ---- AllGather table ----
                nc.gpsimd.collective_compute(
                    "AllGather", AOT.bypass,
                    replica_groups=[list(range(NC))],
                    ins=[zshard[:, :]], outs=[table[:, :]],
                )

                # ---- edge pipeline ----
                blk = 0
                for t in range(TPC):
                    agg = ps_ag.tile([P, D + 2], F32, tag="agg")
                    first = True
                    for (Bn, base, hi) in ((int(B_lo[t]), 0, False),
                                           (int(B_hi[t]), LO, True)):
                        gsl = gring.tile([P, MAXB * R], F32, tag="gsl")
                        nc.gpsimd.dma_gather(
                            out_ap=gsl[:, :Bn * R].rearrange(
                                "p (a d) -> p a d", d=R),
                            in_ap=table[base:NT if hi else LO, :],
                            idxs_ap=idx_t[:, blk * 8: blk * 8 + Bn * 8],
                            num_idxs=Bn * P, num_idxs_reg=Bn * P,
                            elem_size=R, single_packet=False,
                        )
                        for b in range(Bn):
                            j = blk + b
                            # M one-hot from dst_local column
                            m = blkp.tile([P, P], F32, tag="m")
                            nc.vector.tensor_scalar(
                                out=m[:], in0=iota_row[:],
                                scalar1=dstp_t[:, j:j + 1], scalar2=None,
                                op0=AOT.is_equal)
                            # M^T via PE, then s2_dst = (M^T).T @ s2
                            mtp = ps_tr.tile([P, P], F32, tag="tr")
                            nc.tensor.transpose(out=mtp[:], in_=m[:],
                                                identity=ident[:])
                            mts = blkp.tile([P, P], F32, tag="mts")
                            nc.scalar.copy(mts[:], mtp[:])
                            s2d = ps_s2.tile([P, 1], F32, tag="s2d")
                            nc.tensor.matmul(s2d[:], mts[:],
                                             s2sb[:, t:t + 1],
                                             start=True, stop=True)
                            # logits l = s1 + s2d + t ; p = exp(leaky(l))
                            x = blkp.tile([P, 1], F32, tag="x")
                            nc.vector.tensor_scalar(
                                out=x[:],
                                in0=gsl[:, b * R + D + 1: b * R + D + 2],
                                scalar1=s2d[:, 0:1],
                                scalar2=tcol_t[:, j:j + 1],
                                op0=AOT.add, op1=AOT.add)
                            e = blkp.tile([P, 1], F32, tag="e")
                            nc.vector.tensor_scalar(
                                out=e[:], in0=x[:], scalar1=0.01,
                                scalar2=x[:, 0:1],
                                op0=AOT.mult, op1=AOT.max)
                            pv = blkp.tile([P, 1], F32, tag="pv")
                            nc.scalar.activation(pv[:], e[:], ACT.Exp)
                            # Mp = M * p
                            nc.scalar.mul(m[:], m[:], pv[:, 0:1])
                            # aggregate [z|1] -> [z_nb | denom]
                            nc.tensor.matmul(
                                agg[:, 0:D + 1], m[:],
                                gsl[:, b * R: b * R + D + 1],
                                start=first, stop=(hi and b == Bn - 1))
                            first = False
                        blk += Bn

                    # ---- finalize tile ----
                    den = blkp.tile([P, 1], F32, tag="den")
                    nc.vector.tensor_scalar_max(
                        den[:], agg[:, D:D + 1], 1.0e-30)
                    rde = blkp.tile([P, 1], F32, tag="rde")
                    nc.vector.reciprocal(rde[:], den[:])
                    hn = hnew_p.tile([P, D], F32, tag=f"hn{t}")
                    nc.vector.tensor_scalar_mul(
                        hn[:], agg[:, 0:D], rde[:, 0:1])
                    nc.vector.tensor_add(hn[:], hn[:], zi_tiles[t][:])
                    nc.scalar.activation(hn[:], hn[:], ACT.Relu)
                    if layer == L - 1:
                        nc.sync.dma_start(hout[t * P:(t + 1) * P, :], hn[:])
                    h_tiles[t] = hn
                hcur_p, hnew_p = hnew_p, hcur_p
    nc.compile()
    return nc


_CACHE = {}


def kernel(attr, d, src, dst, W0, W1, W2, Wa):
    attr = np.asarray(attr, np.float32)
    d = np.asarray(d, np.float32).reshape(-1)
    src = np.asarray(src).astype(np.int64)
    dst = np.asarray(dst).astype(np.int64)
    W0 = np.asarray(W0, np.float32)
    W1 = np.asarray(W1, np.float32)
    W2 = np.asarray(W2, np.float32)
    Wa = np.asarray(Wa, np.float32)

    B_lo, B_hi, NBLK, idx16, dstp, dcol = preprocess(src, dst, d)

    key = (tuple(B_lo), tuple(B_hi))
    if key not in _CACHE:
        _CACHE[key] = build_nc(B_lo, B_hi, NBLK)
    nc = _CACHE[key]

    # weights: waug[l] = [W1 | W1@wa1 | W1@wa2 | W2]  (D x 2D+2)
    waug = np.zeros((L, D, 2 * D + 2), np.float32)
    for l in range(L):
        wa1 = Wa[l, :D, 0:1]
        wa2 = Wa[l, D:2 * D, 0:1]
        waug[l, :, 0:D] = W1[l]
        waug[l, :, D:D + 1] = W1[l] @ wa1
        waug[l, :, D + 1:D + 2] = W1[l] @ wa2
        waug[l, :, D + 2:] = W2[l]
    waug = np.concatenate([waug[l] for l in range(L)], axis=1)
    c0 = np.array([W0[l, 0, 0] * Wa[l, 2 * D, 0] for l in range(L)], np.float32)
    c0b = np.tile(c0[None, :], (P, 1)).astype(np.float32)

    in_maps = []
    for c in range(NC):
        h0 = np.zeros((SHP, D), np.float32)
        h0[:SH] = attr[c * SH:(c + 1) * SH]
        in_maps.append({
            "h0": h0, "waug": waug, "c0b": c0b,
            "idx16": np.ascontiguousarray(idx16[c]),
            "dstp": np.ascontiguousarray(dstp[c]),
            "dcol": np.ascontiguousarray(dcol[c]),
        })

    res = run_bass_kernel_spmd(nc, in_maps, core_ids=list(range(NC)))
    out = np.empty((N, D), np.float32)
    for c in range(NC):
        out[c * SH:(c + 1) * SH] = res.results[c]["hout"][:SH]
    return out
